# revision 47
# baseline (speedup 1.0000x reference)
"""Trainium2 Bass kernel for nn_ExtraPositionPromptSABottleneck.

Data-parallel over batch B=8 across 8 NeuronCores; each core computes one
batch element's full bottleneck block:

  x1 = silu(bn1(cv1 @ x))            [C=256, N=4096]
  q/k/e = proj(x1); v^T materialized directly via matmul
  S^T[m,n] = k^T q + (rel-pos term)  computed in transposed layout so that
             softmax-exp needs no max subtraction (scores are |s|<40) and
             the output matmul out = v @ attn^T needs no 4096^2 transpose.
  rel-pos:   folded host-side into pmat [C,N] added to the q-side projection.
  softmax:   k_b/e_b biases drop out (softmax row-shift invariance); v_b is
             folded into b2 (attn rows sum to 1 after normalization);
             row sums via ones-vector matmul; normalization deferred to the
             PSUM->SBUF evacuation of the output accumulator (fused mul with
             a PE-broadcast 1/Z tile).
  y = silu(bn2(cv2 @ out_norm)) + x

All tensors are bf16 (PSUM accumulation fp32): bf16 stationary operands
double-buffer LDWEIGHTS in the PE background weight plane (fp32r weights
occupy both planes and serialize), and DVE gets 2x throughput on 16-bit.
SiLU of phase D is computed as u + u*tanh(u) with the 1/2 folded into cv2
host-side so ACT stays on the exp/tanh table set the whole kernel.
"""

import numpy as np
import ml_dtypes

import concourse.bass as bass
import concourse.tile as tile
from concourse import bacc, mybir
from concourse.bass_utils import run_bass_kernel_spmd

f32 = mybir.dt.float32
bf16 = mybir.dt.bfloat16
AF = mybir.ActivationFunctionType
ALU = mybir.AluOpType

B, DIMS, SIZE = 8, 512, 64
C = DIMS // 2              # 256
N = SIZE * SIZE            # 4096
NBLK = 512                 # column block (one PSUM bank of fp32)
NNB = N // NBLK            # 8 n blocks
MB = N // 128              # 32 m blocks
EPS = 1e-5
BF = ml_dtypes.bfloat16


def build_nc():
    nc = bacc.Bacc("TRN2", target_bir_lowering=False, debug=False)

    x_d = nc.dram_tensor("x", [DIMS, N], bf16, kind="ExternalInput")
    cv1_d = nc.dram_tensor("cv1_lhsT", [DIMS, C], bf16, kind="ExternalInput")
    b1_d = nc.dram_tensor("b1", [128, 2], f32, kind="ExternalInput")
    qw_d = nc.dram_tensor("q_lhsT", [C, C], bf16, kind="ExternalInput")
    vw_d = nc.dram_tensor("v_rhs", [C, C], bf16, kind="ExternalInput")
    pm_d = nc.dram_tensor("pmat", [C, N], bf16, kind="ExternalInput")
    cv2_d = nc.dram_tensor("cv2_lhsT", [C, DIMS], bf16, kind="ExternalInput")
    b2_d = nc.dram_tensor("b2", [128, 4], f32, kind="ExternalInput")
    b2f_d = nc.dram_tensor("b2f", [128, 4], f32, kind="ExternalInput")
    ones_d = nc.dram_tensor("ones_col", [128, 1], bf16, kind="ExternalInput")
    ones1_d = nc.dram_tensor("ones_row", [1, 128], bf16, kind="ExternalInput")
    y_d = nc.dram_tensor("y", [DIMS, N], bf16, kind="ExternalOutput")

    with tile.TileContext(nc) as tc:
        with (
            tc.tile_pool(name="wp", bufs=1) as wp,
            tc.tile_pool(name="qp", bufs=1) as qp_pool,
            tc.tile_pool(name="vtp", bufs=1) as vt_pool,
            tc.tile_pool(name="bigbuf", bufs=4) as bigbuf,
            tc.tile_pool(name="xpanels", bufs=4) as xp_pool,
            tc.tile_pool(name="work", bufs=10) as work,
            tc.tile_pool(name="small", bufs=2) as small,
            tc.tile_pool(name="resp", bufs=2) as rp_pool,
            tc.tile_pool(name="esum8p", bufs=2) as e8_pool,
        ):
            # ---- weights / constants ----
            cv1_t = wp.tile([128, 4, C], bf16, tag="cv1_t")
            nc.sync.dma_start(cv1_t[:], cv1_d.rearrange("(k p) m -> p k m", p=128))
            xp_pre = {}
            for nb0 in range(3):
                xt = xp_pool.tile([128, 4, NBLK], bf16, tag="x",
                                  name=f"xp_pre{nb0}")
                # second HW-DGE ring (ACT's, idle at head): overlaps the
                # weight fetches on the sync ring
                nc.scalar.dma_start(
                    xt[:],
                    x_d.rearrange("(k p) n -> p k n", p=128)[:, :, bass.ts(nb0, NBLK)])
                xp_pre[nb0] = xt
            b1_t = wp.tile([128, 2], f32, tag="b1_t")
            nc.sync.dma_start(b1_t[:], b1_d[:])
            qw_t = wp.tile([128, 2, C], bf16, tag="qw_t")
            nc.sync.dma_start(qw_t[:], qw_d.rearrange("(k p) m -> p k m", p=128))
            vw_t = wp.tile([128, 2, C], bf16, tag="vw_t")
            nc.sync.dma_start(vw_t[:], vw_d.rearrange("(k p) m -> p k m", p=128))
            ones_t = wp.tile([128, 1], bf16, tag="ones_t")
            nc.sync.dma_start(ones_t[:], ones_d[:])
            ones1_t = wp.tile([1, 128], bf16, tag="ones1_t")
            nc.sync.dma_start(ones1_t[:], ones1_d[:])
            b2_t = wp.tile([128, 4], f32, tag="b2_t")
            nc.sync.dma_start(b2_t[:], b2_d[:])
            b2f_t = wp.tile([128, 4], f32, tag="b2f_t")
            nc.sync.dma_start(b2f_t[:], b2f_d[:])
            cv2_t = wp.tile([128, 2, DIMS], bf16, tag="cv2_t")
            nc.sync.dma_start(cv2_t[:], cv2_d.rearrange("(k p) m -> p k m", p=128))

            # ---- persistent big tensors ----
            qp_t = [qp_pool.tile([128, N], bf16, tag=f"qp{c}", name=f"qp{c}")
                    for c in range(2)]
            vt_t = vt_pool.tile([128, MB * C], bf16, tag="vt")
            x1_t = [bigbuf.tile([128, N], bf16, tag="big", name=f"x1_{c}")
                    for c in range(2)]
            ke_t = x1_t  # cc^T = x1^T (kw^T qw) x1: x1 itself is the k-side operand

            # =========== Phase A+B: x -> x1 -> q,vT (per n-block) ======
            pm_tiles = {}

            def fetch_pm(nbf):
                t = xp_pool.tile([128, 2, NBLK], bf16, tag="pm",
                                 name=f"pm_{nbf}")
                nc.sync.dma_start(
                    t[:],
                    pm_d.rearrange("(c p) n -> p c n", p=128)[:, :, bass.ts(nbf, NBLK)])
                pm_tiles[nbf] = [t[:, 0, :], t[:, 1, :]]
            fetch_pm(0)
            with tc.tile_pool(name="psAB", bufs=8, space="PSUM") as psAB:
                def emit_qv(nb):
                    # q/vT projections for a finished x1 n-block (runs one
                    # block behind x1 so the PE never waits on the silu)
                    ns = bass.ts(nb, NBLK)
                    for cb in range(2):
                        ps = psAB.tile([128, NBLK], f32, tag="ps", name=f"qps{nb}_{cb}")
                        for kc in range(2):
                            nc.tensor.matmul(ps[:], qw_t[:, kc, bass.ts(cb, 128)],
                                             x1_t[kc][:, ns], start=(kc == 0),
                                             stop=(kc == 1))
                        nc.vector.tensor_add(qp_t[cb][:, ns], ps[:],
                                             pm_tiles[nb][cb][:])
                    # vT tiles (v bias folded into b2: attn rows sum to 1)
                    for sb in range(4):
                        m = nb * 4 + sb
                        msl = bass.ts(nb * 4 + sb, 128)  # columns of x1
                        ps = psAB.tile([128, C], f32, tag="ps", name=f"vps{m}")
                        nc.tensor.matmul(ps[:], x1_t[0][:, msl], vw_t[:, 0, :],
                                         start=True, stop=False)
                        nc.tensor.matmul(ps[:], x1_t[1][:, msl], vw_t[:, 1, :],
                                         start=False, stop=True)
                        # evacuate on ACT (idle in this phase; DVE trailing
                        # these copies was gating the first S matmul)
                        nc.scalar.activation(vt_t[:, bass.ts(m, C)], ps[:],
                                             AF.Identity)

                for nb in range(NNB):
                    ns = bass.ts(nb, NBLK)
                    if nb + 1 < NNB:
                        fetch_pm(nb + 1)
                    if nb + 3 < NNB:
                        xt = xp_pool.tile([128, 4, NBLK], bf16, tag="x",
                                          name=f"xp_pre{nb + 3}")
                        nc.sync.dma_start(
                            xt[:],
                            x_d.rearrange("(k p) n -> p k n",
                                          p=128)[:, :, bass.ts(nb + 3, NBLK)])
                        xp_pre[nb + 3] = xt
                    xt = xp_pre[nb]
                    xp = [xt[:, kc, :] for kc in range(4)]
                    # x1 = silu(cv1' @ x + b1')
                    for cb in range(2):
                        ps = psAB.tile([128, NBLK], f32, tag="ps", name=f"x1ps{nb}_{cb}")
                        for kc in range(4):
                            nc.tensor.matmul(ps[:], cv1_t[:, kc, bass.ts(cb, 128)],
                                             xp[kc], start=(kc == 0), stop=(kc == 3))
                        nc.scalar.activation(x1_t[cb][:, ns], ps[:], AF.Silu,
                                             bias=b1_t[:, cb:cb + 1])
                    if nb > 0:
                        emit_qv(nb - 1)
                # prefetch the exp ACT table while the last q/vT block runs
                # (otherwise the 1.28us table load gates the first real exp)
                dmy = wp.tile([128, 1], bf16, tag="dmy")
                nc.scalar.activation(dmy[:], ones_t[:], AF.Exp)
                emit_qv(NNB - 1)

            # ====== Phase C+D fused: attention, then per-n-block conv2 ======
            out_t = [bigbuf.tile([128, N], bf16, tag="big", name=f"out_{c}")
                     for c in range(2)]
            with (
                tc.tile_pool(name="ps_s", bufs=3, space="PSUM") as ps_s,
                tc.tile_pool(name="ps_o", bufs=2, space="PSUM") as ps_o,
                tc.tile_pool(name="ps_n", bufs=1, space="PSUM") as ps_n,
                tc.tile_pool(name="ps_y", bufs=2, space="PSUM") as ps_y,
            ):
                def make_norm(nb, zrow):
                    ns = bass.ts(nb, NBLK)
                    def norm():
                        # 1/Z on DVE (fast approx, ~18 bits), cast to bf16,
                        # broadcast to all partitions via PE, then normalize
                        # the evacuated accumulators in place (mul reads the
                        # broadcast straight from PSUM).
                        rc = small.tile([1, NBLK], f32, tag="sm",
                                        name=f"rc{nb}")
                        nc.vector.reciprocal_approx_fast(rc[:], zrow[:])
                        rcb = small.tile([1, NBLK], bf16, tag="sm",
                                         name=f"rcb{nb}")
                        nc.vector.tensor_copy(rcb[:], rc[:])
                        bc = ps_y.tile([128, NBLK], f32, tag="yps",
                                       name=f"bc{nb}")
                        nc.tensor.matmul(bc[:], ones1_t[:], rcb[:],
                                         start=True, stop=True)
                        for cb in range(2):
                            nc.vector.tensor_mul(out_t[cb][:, ns],
                                                 out_t[cb][:, ns], bc[:])
                    return norm

                res_tiles = {}

                def make_yd(nb, ob):
                    ns = bass.ts(nb, NBLK)
                    last_nb = (nb == NNB - 1)
                    def yd():
                        if ob == 0:
                            res_tiles[nb] = rp_pool.tile([128, 4, NBLK], bf16,
                                                         tag="res",
                                                         name=f"res{nb}")
                        u = ps_y.tile([128, NBLK], f32, tag="yps",
                                      name=f"u{nb}_{ob}")
                        for kc in range(2):
                            nc.tensor.matmul(u[:],
                                             cv2_t[:, kc, bass.ts(ob, 128)],
                                             out_t[kc][:, ns],
                                             start=(kc == 0), stop=(kc == 1))
                        x2 = x2_tiles[nb][:, ob, :]
                        res = res_tiles[nb][:, ob, :]
                        if last_nb:
                            # no exps remain: switch ACT tables once and use
                            # silu directly -- shortest possible drain chain
                            sl = work.tile([128, NBLK], bf16, tag="wk",
                                           name=f"sl{nb}_{ob}")
                            nc.scalar.activation(sl[:], u[:], AF.Silu,
                                                 bias=b2f_t[:, ob:ob + 1],
                                                 scale=2.0)
                            nc.vector.tensor_add(res, sl[:], x2)
                        else:
                            th = work.tile([128, NBLK], bf16, tag="wk",
                                           name=f"th{nb}_{ob}")
                            nc.scalar.activation(th[:], u[:], AF.Tanh,
                                                 bias=b2_t[:, ob:ob + 1])
                            t1 = work.tile([128, NBLK], bf16, tag="wk",
                                           name=f"t1_{nb}_{ob}")
                            nc.vector.tensor_scalar_add(t1[:], th[:], 1.0)
                            # m = (u + b2) * (1 + th)   [fused on DVE]
                            m = work.tile([128, NBLK], bf16, tag="wk",
                                          name=f"m{nb}_{ob}")
                            nc.vector.scalar_tensor_tensor(
                                m[:], u[:], b2_t[:, ob:ob + 1], t1[:],
                                ALU.add, ALU.mult)
                            nc.vector.tensor_add(res, m[:], x2)
                        if last_nb:
                            # store per-ob: starts the final transfers ~3us
                            # earlier than one batched store at ob==3
                            nc.sync.dma_start(y_d[bass.ts(ob, 128), ns], res)
                        elif ob == 3:
                            nc.sync.dma_start(
                                y_d.rearrange("(k p) n -> p k n", p=128)[:, :, ns],
                                res_tiles[nb][:])
                    return yd

                pending = []   # deferred closures, drained 1/mb-slot
                x2_tiles = {}

                for nb in range(NNB):
                    ns = bass.ts(nb, NBLK)
                    ops = [ps_o.tile([128, NBLK], f32, tag="oacc",
                                     name=f"oacc{nb}_{cb}") for cb in range(2)]
                    sps = ps_n.tile([128, NBLK], f32, tag="nacc", name=f"nacc{nb}")
                    def emit_out(mb, es):
                        last = (mb == MB - 1)
                        nc.tensor.matmul(ops[0][:], vt_t[:, mb * C:mb * C + 128],
                                         es, start=(mb == 0), stop=last)
                        nc.tensor.matmul(ops[1][:],
                                         vt_t[:, mb * C + 128:mb * C + 256],
                                         es, start=(mb == 0), stop=last)

                    sum_q = []     # sums matmuls deferred >=1 bt so the PE
                    esum4_prev = None   # never waits on the DVE esum chain
                    esum8_prev = None
                    for bt in range(MB // 4):
                        if sum_q:
                            sum_q.pop(0)()
                        mbs = list(range(4 * bt, 4 * bt + 4))
                        sts = {}
                        for mb in mbs:   # kc-inner: each tile stops earlier
                            sts[mb] = ps_s.tile([128, NBLK], f32, tag="st",
                                                name=f"st{nb}_{mb}")
                            for kc in range(2):
                                nc.tensor.matmul(sts[mb][:],
                                                 ke_t[kc][:, bass.ts(mb, 128)],
                                                 qp_t[kc][:, ns],
                                                 start=(kc == 0), stop=(kc == 1))
                        ess = {}
                        esums = []
                        for j in range(2):
                            for mb in mbs[2 * j:2 * j + 2]:
                                es = work.tile([128, NBLK], bf16, tag="wk",
                                               name=f"es{nb}_{mb}")
                                nc.scalar.activation(es[:], sts[mb][:], AF.Exp)
                                ess[mb] = es
                            esum = work.tile([128, NBLK], bf16, tag="wk",
                                             name=f"esum{nb}_{bt}_{j}")
                            nc.vector.tensor_add(esum[:], ess[mbs[2 * j]][:],
                                                 ess[mbs[2 * j + 1]][:])
                            esums.append(esum)
                        esum4 = work.tile([128, NBLK], bf16, tag="wk",
                                          name=f"esum4_{nb}_{bt}")
                        nc.vector.tensor_add(esum4[:], esums[0][:], esums[1][:])
                        if bt % 2 == 1:
                            # esum8/esum16 live across bts -> dedicated pool
                            # (the shared work-ring would recycle their
                            # buffers before the deferred reads)
                            esum8 = e8_pool.tile([128, NBLK], bf16, tag="e8",
                                                 name=f"esum8_{nb}_{bt // 2}")
                            nc.vector.tensor_add(esum8[:], esum4_prev[:],
                                                 esum4[:])
                            if bt % 4 == 1:
                                esum8_prev = esum8
                            else:
                                esum16 = e8_pool.tile([128, NBLK], bf16,
                                                      tag="e16",
                                                      name=f"esum16_{nb}_{bt // 4}")
                                nc.vector.tensor_add(esum16[:], esum8_prev[:],
                                                     esum8[:])
                                def sums_mm(e=esum16, j=bt // 4):
                                    nc.tensor.matmul(sps[0:1, :], ones_t[:],
                                                     e[:], start=(j == 0),
                                                     stop=(j == MB // 16 - 1))
                                sum_q.append(sums_mm)
                        else:
                            esum4_prev = esum4
                        for mb in mbs:
                            emit_out(mb, ess[mb][:])
                        if bt in (1, 2, 3, 4, 5) and pending:
                            pending.pop(0)()
                    sum_q.pop(0)()

                    # row-sums -> SBUF row (PE rhs must live in SBUF);
                    # evacuate out accumulators inline (deferred norm would
                    # race the next n-block's reuse of the ps_o banks)
                    zrow = small.tile([1, NBLK], f32, tag="sm", name=f"zrow{nb}")
                    nc.scalar.activation(zrow[:], sps[0:1, :], AF.Identity)
                    for cb in range(2):
                        nc.scalar.activation(out_t[cb][:, ns], ops[cb][:],
                                             AF.Identity)
                    x2t = xp_pool.tile([128, 4, NBLK], bf16, tag="x",
                                       name=f"x2_{nb}")
                    nc.sync.dma_start(
                        x2t[:],
                        x_d.rearrange("(k p) n -> p k n", p=128)[:, :, ns])
                    x2_tiles[nb] = x2t
                    if nb < NNB - 1:
                        pending.append(make_norm(nb, zrow))
                        for ob in range(4):
                            pending.append(make_yd(nb, ob))
                while pending:
                    pending.pop(0)()

                # final n-block drain in two 256-col halves: the serial
                # norm->conv2->silu->store chain halves its stage sizes and
                # the two halves pipeline across PE/ACT/DVE/DMA
                nb = NNB - 1
                ns0 = nb * NBLK
                HB = NBLK // 2
                for h in range(2):
                    hs = slice(h * HB, (h + 1) * HB)
                    nsh = slice(ns0 + h * HB, ns0 + (h + 1) * HB)
                    zrh = small.tile([1, HB], f32, tag=f"szr{h}",
                                     name=f"zrh{h}")
                    nc.scalar.activation(zrh[:], sps[0:1, hs], AF.Identity)
                    for cb in range(2):
                        nc.scalar.activation(out_t[cb][:, nsh],
                                             ops[cb][:, hs], AF.Identity)
                    rch = small.tile([1, HB], f32, tag=f"src{h}",
                                     name=f"rch{h}")
                    nc.vector.reciprocal_approx_fast(rch[:], zrh[:])
                    rcbh = small.tile([1, HB], bf16, tag=f"srb{h}",
                                      name=f"rcbh{h}")
                    nc.vector.tensor_copy(rcbh[:], rch[:])
                    bch = ps_y.tile([128, HB], f32, tag="yps",
                                    name=f"bch{h}")
                    nc.tensor.matmul(bch[:], ones1_t[:], rcbh[:],
                                     start=True, stop=True)
                    for cb in range(2):
                        nc.vector.tensor_mul(out_t[cb][:, nsh],
                                             out_t[cb][:, nsh], bch[:])
                    for ob in range(4):
                        uh = ps_y.tile([128, HB], f32, tag="yps",
                                       name=f"uh{h}_{ob}")
                        for kc in range(2):
                            nc.tensor.matmul(uh[:],
                                             cv2_t[:, kc, bass.ts(ob, 128)],
                                             out_t[kc][:, nsh],
                                             start=(kc == 0), stop=(kc == 1))
                        slh = work.tile([128, HB], bf16, tag="wk",
                                        name=f"slh{h}_{ob}")
                        nc.scalar.activation(slh[:], uh[:], AF.Silu,
                                             bias=b2f_t[:, ob:ob + 1],
                                             scale=2.0)
                        resh = work.tile([128, HB], bf16, tag="wk",
                                         name=f"resh{h}_{ob}")
                        nc.vector.tensor_add(resh[:],
                                             slh[:],
                                             x2_tiles[nb][:, ob, hs])
                        eng = nc.sync if (ob % 2 == 0) else nc.scalar
                        eng.dma_start(y_d[bass.ts(ob, 128), nsh], resh[:])

    nc.compile()
    return nc


def prep_inputs(inputs):
    """Host-side folding of BN + weight layouts. Returns the shared in_map."""
    i = {k: np.asarray(v, dtype=np.float32) if np.asarray(v).dtype == np.float32
         else np.asarray(v) for k, v in inputs.items()}
    s1 = i["bn1_g"] / np.sqrt(i["bn1_v"] + EPS)
    cv1w = i["cv1_w"] * s1[:, None]                       # [C, DIMS]
    b1 = i["bn1_b"] - i["bn1_m"] * s1                     # [C]
    s2 = i["bn2_g"] / np.sqrt(i["bn2_v"] + EPS)
    cv2w = 0.5 * i["cv2_w"] * s2[:, None]                 # [DIMS, C] (1/2 for tanh-silu)
    # v bias folded: out rows of attn sum to 1 -> +v_b per channel -> conv2
    b2 = 0.5 * (i["bn2_b"] - i["bn2_m"] * s2) + cv2w @ i["v_b"]  # [DIMS]

    pos = (i["rel_h"] + i["rel_w"]).reshape(C, N).astype(np.float64)
    pmat = (i["e_w"].astype(np.float64).T @ pos
            + (i["k_w"].astype(np.float64).T @ i["q_b"].astype(np.float64))[:, None]
            ).astype(np.float32)

    bf = lambda a: np.ascontiguousarray(a).astype(BF)
    return {
        "cv1_lhsT": bf(cv1w.T),                           # [DIMS, C]
        "b1": np.ascontiguousarray(b1.reshape(2, 128).T),
        "q_lhsT": bf((i["q_w"].astype(np.float64).T
                      @ i["k_w"].astype(np.float64)).astype(np.float32)),
        "v_rhs": bf(i["v_w"].T),                          # [C, C]: v_rhs[ci,c]
        "pmat": bf(pmat),
        "cv2_lhsT": bf(cv2w.T),                           # [C, DIMS]
        "b2": np.ascontiguousarray(b2.reshape(4, 128).T),
        "b2f": np.ascontiguousarray((2.0 * b2).reshape(4, 128).T),
        "ones_col": np.ones((128, 1), BF),
        "ones_row": np.ones((1, 128), BF),
    }


_NC = None


def run(inputs, trace=False):
    global _NC
    if _NC is None:
        _NC = build_nc()
    shared = prep_inputs(inputs)
    x = np.asarray(inputs["x"], dtype=np.float32)  # [B, DIMS, SIZE, SIZE]
    in_maps = []
    for b in range(B):
        m = dict(shared)
        m["x"] = np.ascontiguousarray(x[b].reshape(DIMS, N)).astype(BF)
        in_maps.append(m)
    res = run_bass_kernel_spmd(_NC, in_maps, list(range(B)), trace=trace)
    out = np.stack([np.asarray(res.results[b]["y"]).astype(np.float32)
                    .reshape(DIMS, SIZE, SIZE)
                    for b in range(B)], axis=0)
    return out, res.exec_time_ns


def kernel(**inputs) -> np.ndarray:
    out, _ = run(inputs, trace=False)
    return out
